# revision 34
# baseline (speedup 1.0000x reference)
"""Trainium2 Bass kernel for nn_Attention_74732430950411.

Single-query multi-head attention with RoPE on keys/values, batch=128,
S=1024, VO=QK=512, H=8. Data-parallel over batch across 8 NeuronCores
(16 batch rows per core); weights replicated.

Algebraic restructuring (validated vs reference to ~1.5e-4 absmax, output
scale ~4.4):
  - Wq/Wk folded on host into M_h = Wk_h.T @ Wq_h / sqrt(HD); scores are
    computed as  scores[b,h,s] = sum_d  K[b,s,d] * cos/sin[s,d] * w[b,h,d]
    which removes the [B*S,512]x[512,512] K-projection entirely.
  - rotate_half is implemented by chunk-index cross-wiring of the matmul
    accumulation (sigma = (2,3,0,1)) with the sign baked into the host-side
    sin table (chunks 2,3 negated) -- no data movement.
  - The V-projection is applied after the attention sum: out_h = Wv_h @
    (attn @ rope(V)), removing the other big projection.

Per-core work is memory-bound: stream keys+states (67MB fp32), cast to
bf16 during DMA (SWDGE), one batched xbar-transpose per K row (bf16
SBUF->SBUF), two elementwise table-muls per tensor on DVE, small PE
matmuls. s is indexed as s = 8p + si (p = SBUF partition, si = 0..7) so
each partition's DMA slice is 16KB contiguous.

Batch rows are packed 4-per-128-partition-group ("quads") at partition
bases {0,32,64,96} (row = 32*(b%4) + h) because engine SBUF access
requires 32-aligned start partitions; matmul tile_position places PSUM
writes at the same bases. The main loop pipelines per quad:
K-stream -> scores -> softmax -> transpose -> V-stream -> ctx, with
V-lanes emitted 4 slots behind K-lanes, so DMA, DVE and PE stay
concurrently busy across quads. Softmax normalization is folded into the
attention transpose as a PE matmul against diag(1/rowsum).
"""

import os
import numpy as np
import ml_dtypes

BF = ml_dtypes.bfloat16

B, S, D, H, HD = 128, 1024, 512, 8, 64
NCORES = 8
BL = B // NCORES          # 16 batch rows per core
SC = S // 128             # 8 s-slots per partition
DC = D // 128             # 4 d-chunks
NQ = BL // 4              # 4 quads of 4 batch-lanes

_cache = {}


def _rope_tables():
    inv = 1.0 / (10000.0 ** (np.arange(0, D, 2, dtype=np.float32) / D))
    t = np.arange(S, dtype=np.float32)
    freqs = np.einsum("i,j->ij", t, inv)
    emb = np.concatenate([freqs, freqs], axis=-1)          # [S, D]
    return np.cos(emb).astype(np.float32), np.sin(emb).astype(np.float32)


def _build_program(reps=1):
    """Build + compile the SPMD bass program (once per process).

    reps>1 emits the whole compute body multiple times inside one NEFF --
    used only for timing (amortizes the ~86ms axon launch overhead).
    """
    key = ("nc", reps)
    if key in _cache:
        return _cache[key]

    from contextlib import ExitStack
    import concourse.tile as tile
    from concourse import bacc, mybir
    from concourse.masks import make_identity

    F32 = mybir.dt.float32
    BF16 = mybir.dt.bfloat16
    EXP = mybir.ActivationFunctionType.Exp

    nc = bacc.Bacc("TRN2", target_bir_lowering=False, debug=False)

    keys_d = nc.dram_tensor("keys", [BL, S, D], F32, kind="ExternalInput").ap()
    states_d = nc.dram_tensor("states", [BL, S, D], F32, kind="ExternalInput").ap()
    coskt_d = nc.dram_tensor("coskt", [128, SC, DC, 128], BF16, kind="ExternalInput").ap()
    sinkt_d = nc.dram_tensor("sinkt", [128, SC, DC, 128], BF16, kind="ExternalInput").ap()
    cosv_d = nc.dram_tensor("cosv", [128, SC, D], BF16, kind="ExternalInput").ap()
    sinv_d = nc.dram_tensor("sinv", [128, SC, D], BF16, kind="ExternalInput").ap()
    wq_d = nc.dram_tensor("wq", [D, D], F32, kind="ExternalInput").ap()
    wk_d = nc.dram_tensor("wk", [D, D], F32, kind="ExternalInput").ap()
    wvt_d = nc.dram_tensor("wvt", [128, DC, D], BF16, kind="ExternalInput").ap()
    wot_d = nc.dram_tensor("wot", [128, DC, D], BF16, kind="ExternalInput").ap()
    xtbf_d = nc.dram_tensor("xtbf", [128, DC, BL], BF16, kind="ExternalInput").ap()
    xtf_d = nc.dram_tensor("xtf", [128, DC, BL], F32, kind="ExternalInput").ap()
    yt_d = nc.dram_tensor("yt", [128, DC, BL], F32, kind="ExternalOutput").ap()

    SIGMA = (2, 3, 0, 1)  # rotate_half chunk wiring (sign in sin tables)

    with tile.TileContext(nc) as tc, ExitStack() as ctx:
        const = ctx.enter_context(tc.tile_pool(name="const", bufs=1))

        # --- constants / tables ---
        coskt = const.tile([128, SC, DC, 128], BF16)
        sinkt = const.tile([128, SC, DC, 128], BF16)
        cosv = const.tile([128, SC, D], BF16)
        sinv = const.tile([128, SC, D], BF16)
        wvt = const.tile([128, DC, D], BF16)
        wot = const.tile([128, DC, D], BF16)
        xtbf = const.tile([128, DC, BL], BF16)
        xtf = const.tile([128, DC, BL], F32)
        ident = const.tile([128, 128], BF16)
        nc.sync.dma_start(xtbf[:], xtbf_d)
        make_identity(nc, ident[:])

        w_sb = const.tile([128, DC, BL, H], BF16)        # q-tilde per (chunk,b,h)
        attn_q = [const.tile([128, S], BF16, name=f"attnq{g}", tag=f"attnq{g}")
                  for g in range(2)]
        sums_sb = const.tile([128, NQ], F32)
        recip_sb = const.tile([128, NQ], F32)
        attnT = const.tile([128, SC, NQ, 128], BF16)     # [q, si, g, row]
        ctx_q = [const.tile([128, D], BF16, name=f"ctxq{g}", tag=f"ctxq{g}")
                 for g in range(2)]
        ctxT = const.tile([128, DC, NQ, 128], BF16)      # [d-in-chunk, dc, g, row]
        ot_sb = const.tile([128, DC, BL], BF16)
        y_sb = const.tile([128, DC, BL], F32)

        # unused rows (row % 32 >= 8) hold garbage; keep it finite
        nc.gpsimd.memset(sums_sb[:], 1.0)
        for g in range(2):
            nc.gpsimd.memset(attn_q[g][:], 0.0)
            nc.gpsimd.memset(ctx_q[g][:], 0.0)

        for _rep in range(reps):
            # --- phase 1: L_h = Wq_h.T @ Wk_h (= M_h.T * 8), then
            #     w = q-tilde/8 = (L.T @ x)/8, all on device ---
            wq_sb = const.tile([64, H, D], BF16, name="wq_sb")
            wk_sb = const.tile([64, H, D], BF16, name="wk_sb")
            nc.gpsimd.dma_start(wq_sb[:],
                                wq_d.rearrange("(h i) d -> i h d", i=64))
            nc.gpsimd.dma_start(wk_sb[:],
                                wk_d.rearrange("(h i) d -> i h d", i=64))
            nc.sync.dma_start(coskt[:], coskt_d)
            nc.sync.dma_start(sinkt[:], sinkt_d)

            with tc.tile_pool(name="qps", bufs=2, space="PSUM") as qps, \
                 tc.tile_pool(name="qos", bufs=2, space="PSUM") as qos:
                for h in range(H):
                    msl = const.tile([128, DC, D], BF16, name="msl",
                                     tag="msl", bufs=2)
                    for mc in range(DC):
                        pm = qps.tile([128, 512], F32)
                        nc.tensor.matmul(pm[:],
                                         wq_sb[:, h, 128 * mc:128 * (mc + 1)],
                                         wk_sb[:, h, :],
                                         start=True, stop=True)
                        nc.scalar.copy(msl[:, mc, :], pm[:])
                    for c in range(DC):
                        pq = qos.tile([128, 512], F32)
                        for dk in range(DC):
                            nc.tensor.matmul(pq[:, 0:BL],
                                             msl[:, dk, 128 * c:128 * (c + 1)],
                                             xtbf[:, dk, :],
                                             start=(dk == 0), stop=(dk == DC - 1))
                        nc.scalar.mul(w_sb[:, c, :, h], pq[:, 0:BL], 0.125)

            # --- main loop: software-pipelined interleave of K(t) and V(t-4) ---
            with tc.tile_pool(name="kna", bufs=2) as knp, \
                 tc.tile_pool(name="ktp", bufs=2) as ktp, \
                 tc.tile_pool(name="kcp", bufs=2) as kcp, \
                 tc.tile_pool(name="vna", bufs=2) as vnp, \
                 tc.tile_pool(name="vcp", bufs=2) as vcp, \
                 tc.tile_pool(name="qtm", bufs=1) as qtm, \
                 tc.tile_pool(name="sps", bufs=1, space="PSUM") as sps, \
                 tc.tile_pool(name="tps", bufs=2, space="PSUM") as tps, \
                 tc.tile_pool(name="pqs", bufs=2, space="PSUM") as pqs:
                ps_cur = [None]
                pq_cur = [None]

                def emit_k(b):
                    g, j = divmod(b, 4)
                    base = 32 * j
                    if j == 0:
                        ps_cur[0] = sps.tile([128, S], F32, name=f"ps{g}",
                                             tag="ps")
                    ps = ps_cur[0]
                    knat = knp.tile([128, SC, D], BF16, name="knat")
                    nc.gpsimd.dma_start(
                        knat[:],
                        keys_d[b].rearrange("(p si) d -> p si d", p=128))
                    kt = ktp.tile([128, SC, DC, 128], BF16, name="kt")
                    nc.sync.dma_start_transpose(
                        out=kt[:].rearrange("p si dc q -> p (si dc) q"),
                        in_=knat[:].rearrange("p si d -> p (si d)"))
                    kc = kcp.tile([128, SC, DC, 128], BF16, name="kc", tag="kc")
                    ks = kcp.tile([128, SC, DC, 128], BF16, name="ks", tag="ks")
                    for nh in range(2):
                        sl = slice(512 * nh, 512 * (nh + 1))
                        hs = slice(4 * nh, 4 * nh + 4)
                        nc.vector.tensor_mul(kc[:, hs], kt[:, hs], coskt[:, hs])
                        nc.vector.tensor_mul(ks[:, hs], kt[:, hs], sinkt[:, hs])
                        for c in range(DC):
                            nc.tensor.matmul(ps[base:base + 8, sl],
                                             w_sb[:, c, b, :], kc[:, hs, c, :],
                                             start=(c == 0), stop=False,
                                             tile_position=(0, base))
                        for c in range(DC):
                            nc.tensor.matmul(ps[base:base + 8, sl],
                                             w_sb[:, SIGMA[c], b, :],
                                             ks[:, hs, c, :],
                                             start=False, stop=(c == DC - 1),
                                             tile_position=(0, base))
                    nc.scalar.activation(
                        attn_q[g % 2][base:base + 8, :], ps[base:base + 8, :],
                        EXP, accum_out=sums_sb[base:base + 8, g:g + 1])

                def emit_softmax(g):
                    nc.vector.reciprocal(recip_sb[:, g:g + 1],
                                         sums_sb[:, g:g + 1])
                    # transpose-and-normalize in one PE op: out = attn.T @ diag(r)
                    diag_t = qtm.tile([128, 128], BF16, name="diag_t",
                                      tag="diag", bufs=2)
                    nc.vector.tensor_scalar_mul(diag_t[:], ident[:],
                                                recip_sb[:, g:g + 1])
                    for si in range(SC):
                        pt = tps.tile([128, 128], F32, name="pt", tag="pt")
                        nc.tensor.matmul(pt[:],
                                         attn_q[g % 2][:, 128 * si:128 * (si + 1)],
                                         diag_t[:], start=True, stop=True)
                        nc.scalar.copy(attnT[:, si, g, :], pt[:])

                def emit_v(b):
                    g, j = divmod(b, 4)
                    base = 32 * j
                    if j == 0:
                        pq_cur[0] = pqs.tile([128, 2 * D], F32, name=f"pq{g}",
                                             tag="pq")
                    pq = pq_cur[0]
                    vnat = vnp.tile([128, SC, D], BF16, name="vnat")
                    nc.gpsimd.dma_start(
                        vnat[:],
                        states_d[b].rearrange("(p si) d -> p si d", p=128))
                    vc = vcp.tile([128, SC, D], BF16, name="vc", tag="vc")
                    vs = vcp.tile([128, SC, D], BF16, name="vs", tag="vs")
                    for vh in range(2):
                        vsl = slice(4 * vh, 4 * vh + 4)
                        nc.vector.tensor_mul(vc[:, vsl], vnat[:, vsl], cosv[:, vsl])
                        nc.vector.tensor_mul(vs[:, vsl], vnat[:, vsl], sinv[:, vsl])
                    for si in range(SC):
                        lhs = attnT[:, si, g, base:base + 8]
                        nc.tensor.matmul(pq[base:base + 8, 0:D], lhs,
                                         vc[:, si, :], start=(si == 0),
                                         stop=(si == SC - 1),
                                         tile_position=(0, base))
                        nc.tensor.matmul(pq[base:base + 8, D:2 * D], lhs,
                                         vs[:, si, :], start=(si == 0),
                                         stop=(si == SC - 1),
                                         tile_position=(0, base))
                    # ctx = P + rot(Q)  (signs already in sinv)
                    qt = qtm.tile([128, D], F32, name="qt")
                    r = slice(base, base + 8)
                    nc.scalar.copy(qt[r, :], pq[r, D:2 * D])
                    nc.vector.tensor_add(ctx_q[g % 2][r, 0:256], pq[r, 0:256],
                                         qt[r, 256:D])
                    nc.vector.tensor_add(ctx_q[g % 2][r, 256:D], pq[r, 256:D],
                                         qt[r, 0:256])

                def emit_ctx_t(g):
                    for dc in range(DC):
                        pt = tps.tile([128, 128], BF16, name="pt")
                        nc.tensor.transpose(
                            pt[:], ctx_q[g % 2][:, 128 * dc:128 * (dc + 1)],
                            ident[:])
                        nc.scalar.copy(ctxT[:, dc, g, :], pt[:])

                for t in range(BL + 4):
                    if t == 1:
                        nc.sync.dma_start(cosv[:], cosv_d)
                        nc.sync.dma_start(sinv[:], sinv_d)
                    if t == BL - 2:
                        nc.sync.dma_start(wvt[:], wvt_d)
                        nc.sync.dma_start(wot[:], wot_d)
                        nc.sync.dma_start(xtf[:], xtf_d)
                    v = t - 4
                    if 0 <= v < BL:
                        emit_v(v)
                        if v % 4 == 3:
                            emit_ctx_t(v // 4)
                    if t < BL:
                        emit_k(t)
                        if t % 4 == 3:
                            emit_softmax(t // 4)

            # --- tail: out = Wo @ (Wv_h @ ctx) + x ---
            with tc.tile_pool(name="ops", bufs=4, space="PSUM") as ops, \
                 tc.tile_pool(name="yps", bufs=2, space="PSUM") as yps:
                ctxr = ctxT[:].rearrange("p dc g (j r) -> p dc g j r", r=32)
                for hp in range(4):
                    po = ops.tile([128, 512], F32)
                    for hh in range(2):
                        h = 2 * hp + hh
                        out_sl = po[64 * hh:64 * hh + 64, 0:BL]
                        for dc in range(DC):
                            nc.tensor.matmul(out_sl,
                                             wvt[:, dc, 64 * h:64 * h + 64],
                                             ctxr[:, dc, :, :, h],
                                             start=(dc == 0), stop=(dc == DC - 1),
                                             tile_position=(0, 64 * hh))
                    nc.scalar.copy(ot_sb[:, hp, :], po[:, 0:BL])
                for mc in range(DC):
                    py = yps.tile([128, 512], F32)
                    for kc_ in range(DC):
                        nc.tensor.matmul(py[:, 0:BL],
                                         wot[:, kc_, 128 * mc:128 * (mc + 1)],
                                         ot_sb[:, kc_, :],
                                         start=(kc_ == 0), stop=(kc_ == DC - 1))
                    nc.vector.tensor_add(y_sb[:, mc, :], py[:, 0:BL],
                                         xtf[:, mc, :])
                nc.sync.dma_start(yt_d, y_sb[:])

    nc.compile()
    _cache[key] = nc
    return nc


def _host_prep(x, keys, states, Wq, Wk, Wv, Wo):
    """Shared tables + per-core input maps.  s = 8p + si."""
    cos, sin = _rope_tables()
    sgn = np.ones((1, D), np.float32)
    sgn[0, 256:] = -1.0
    sin_s = sin * sgn

    # cos[s, d] with s = 8p + si, d = 128*dc + a
    # coskt[a, si, dc, p(=q)] ; cosv[p, si, d]
    cos_p = cos.reshape(128, SC, D)                       # [p, si, d]
    sin_p = sin_s.reshape(128, SC, D)
    coskt = np.ascontiguousarray(
        cos_p.reshape(128, SC, DC, 128).transpose(3, 1, 2, 0)).astype(BF)
    sinkt = np.ascontiguousarray(
        sin_p.reshape(128, SC, DC, 128).transpose(3, 1, 2, 0)).astype(BF)
    cosv = np.ascontiguousarray(cos_p).astype(BF)
    sinv = np.ascontiguousarray(sin_p).astype(BF)

    wvt = np.ascontiguousarray(
        Wv.T.reshape(DC, 128, D).transpose(1, 0, 2)).astype(BF)
    wot = np.ascontiguousarray(
        Wo.T.reshape(DC, 128, D).transpose(1, 0, 2)).astype(BF)

    in_maps = []
    for core in range(NCORES):
        bs = slice(core * BL, (core + 1) * BL)
        xs = x[bs]                                        # [BL, D]
        xt = np.ascontiguousarray(xs.T.reshape(DC, 128, BL).transpose(1, 0, 2))
        in_maps.append({
            "keys": np.ascontiguousarray(keys[bs]),
            "states": np.ascontiguousarray(states[bs]),
            "coskt": coskt, "sinkt": sinkt, "cosv": cosv, "sinv": sinv,
            "wq": np.ascontiguousarray(Wq), "wk": np.ascontiguousarray(Wk),
            "wvt": wvt, "wot": wot,
            "xtbf": xt.astype(BF), "xtf": xt.astype(np.float32),
        })
    return in_maps


def run_on_device(in_maps, reps=1):
    from concourse.bass_utils import run_bass_kernel_spmd
    nc = _build_program(reps)
    res = run_bass_kernel_spmd(nc, in_maps, core_ids=list(range(NCORES)))
    return res


def kernel(x, keys, states, Wq, Wk, Wv, Wo):
    x = np.asarray(x, dtype=np.float32)
    keys = np.asarray(keys, dtype=np.float32)
    states = np.asarray(states, dtype=np.float32)
    in_maps = _host_prep(x, keys, states,
                         np.asarray(Wq, np.float32), np.asarray(Wk, np.float32),
                         np.asarray(Wv, np.float32), np.asarray(Wo, np.float32))
    res = run_on_device(in_maps)
    outs = []
    for core in range(NCORES):
        yt = np.asarray(res.results[core]["yt"])          # [128, DC, BL]
        outs.append(yt.transpose(2, 1, 0).reshape(BL, D))
    return np.concatenate(outs, axis=0).reshape(B, 1, D).astype(np.float32)


if __name__ == "__main__":
    rng = np.random.default_rng(0)
    out = kernel(
        x=rng.standard_normal((B, D)).astype(np.float32),
        keys=rng.standard_normal((B, S, D)).astype(np.float32),
        states=rng.standard_normal((B, S, D)).astype(np.float32),
        Wq=(rng.standard_normal((D, D)) * 0.02).astype(np.float32),
        Wk=(rng.standard_normal((D, D)) * 0.02).astype(np.float32),
        Wv=(rng.standard_normal((D, D)) * 0.02).astype(np.float32),
        Wo=(rng.standard_normal((D, D)) * 0.02).astype(np.float32),
    )
    print("out", out.shape, out.dtype, np.abs(out).max())
